# revision 87
# baseline (speedup 1.0000x reference)
"""MultiHeadAttention Trainium2 kernel: mask-aware balanced sharding.

Work assignment (computed from the runtime src_batch_lens):
  Sort batches by key-tile count d_b = ceil(len_b/128), descending ranks r0..r7.
  Each core runs three attention "slots" (SPMD-uniform shapes):
    slot A: 512 queries x T_A key-tiles  (T_A = d[r0]; batches r0, r1)
    slot B: 1024 queries x T_B key-tiles (T_B = d[r2]; batches r2..r5)
    slot C: 512 queries x T_C key-tiles  (T_C = d[r6]; batches r6, r7)
  Core c gets one q-chunk of one batch per slot; the host stages per-core
  inputs (transposed bf16 x slices, truncated to T_s*128 keys -- the unmasked
  keys are a prefix) and unscrambles the outputs, so the program is uniform
  across cores and the masked-key work is skipped nearly load-balanced
  (sum T_s = 30 key-tiles/core here vs 32 dense). The per-slot mask bias
  (-1e9 -> exp==0) handles partial tiles and padded jobs.

Per-slot structure: projections with the contraction dim on partitions,
scores^T per (head, k-tile) with exp on ScalarE (scale + mask bias fused,
reading PSUM), P@V with a ones column on V giving the softmax denominator for
free, output projection from ctx^T. Slots A/C process HEAD PAIRS per step
(heads 2hp/2hp+1 live in the two partition halves of one kT/qT group), so
every exp instruction covers the full [128 x 1024] PSUM pair. A and C share
SBUF via common pool tags (valid since T_A >= T_C).

Scheduling: a work queue of 512-col projection groups and output-projection
groups is popped inside the attention loops at a cadence matched to the
ACT-vs-PE step deficit (~330ns/step), so the PE stays busy behind the exp
stream; pop order respects every K-tile group's first-use deadline. The
softmax flush runs off the PE in steady state (DVE copy+reciprocal, Pool
partition-broadcast + normalize); the last flushes use a lower-latency
PE-broadcast + DVE path. The tail output projection is staged (mt0-2 partials
accumulate during the final head-pair; only the mt3 matmuls remain after the
last flush). DMA order is tuned so the first projections never wait.
"""

import numpy as np
import ml_dtypes

import concourse.bass as bass  # noqa: F401
import concourse.tile as tile
from concourse import bacc, mybir
from concourse._compat import get_trn_type
from concourse.bass_utils import run_bass_kernel_spmd

B, S, D = 8, 2048, 512
H, DH = 8, 64
P = 128
NDT = D // P  # 4 tiles over the model dim
F32 = mybir.dt.float32
BF16 = mybir.dt.bfloat16
NEG = -1.0e9

last_results = None


def _build_program(T_A, T_B, T_C):
    nc = bacc.Bacc(get_trn_type() or "TRN2", target_bir_lowering=False)

    slots = [("A", 512, T_A), ("B", 1024, T_B), ("C", 512, T_C)]

    d_in = {}
    for s, qc, t in slots:
        d_in[f"xq{s}"] = nc.dram_tensor(f"xq{s}", (P, NDT, qc), BF16, kind="ExternalInput")
        d_in[f"xk{s}"] = nc.dram_tensor(f"xk{s}", (P, NDT, t * P), BF16, kind="ExternalInput")
        d_in[f"xv{s}"] = nc.dram_tensor(f"xv{s}", (P, NDT, t * P), BF16, kind="ExternalInput")
        d_in[f"mask{s}"] = nc.dram_tensor(f"mask{s}", (P, t), F32, kind="ExternalInput")
    for w in ("wq", "wk", "wv", "wo"):
        d_in[w] = nc.dram_tensor(w, (P, NDT, D), BF16, kind="ExternalInput")
    d_in["bqT"] = nc.dram_tensor("bqT", (P, NDT), F32, kind="ExternalInput")
    d_in["bkT"] = nc.dram_tensor("bkT", (P, NDT), F32, kind="ExternalInput")
    d_in["bvb"] = nc.dram_tensor("bvb", (P, D), BF16, kind="ExternalInput")
    d_in["bob"] = nc.dram_tensor("bob", (P, D), BF16, kind="ExternalInput")
    out_d = nc.dram_tensor("out", (P, 16, D), BF16, kind="ExternalOutput")

    Exp = mybir.ActivationFunctionType.Exp
    MUL = mybir.AluOpType.mult
    ADD = mybir.AluOpType.add

    qT, kT, vv, ctxT, msk = {}, {}, {}, {}, {}
    xq, xk, xv = {}, {}, {}

    with tile.TileContext(nc) as tc:
        with tc.tile_pool(name="persist", bufs=1) as pp, \
             tc.tile_pool(name="xin", bufs=1) as xp:

            def load_x(s, qc, t):
                sh = "AC" if s in ("A", "C") else "B"
                xq[s] = xp.tile([P, NDT, qc], BF16, tag=f"xq{sh}", name="xq")
                nc.sync.dma_start(xq[s][:], d_in[f"xq{s}"][:])
                nk = t * P
                h1 = min(nk, (nk // 2 + P - 1) // P * P) if t > 1 else nk
                xk[s] = xp.tile([P, NDT, nk], BF16, tag=f"xk{sh}", name="xk")
                nc.sync.dma_start(xk[s][:, :, 0:h1], d_in[f"xk{s}"][:, :, 0:h1])
                if h1 < nk:
                    nc.sync.dma_start(xk[s][:, :, h1:nk], d_in[f"xk{s}"][:, :, h1:nk])
                xv[s] = xp.tile([P, NDT, nk], BF16, tag=f"xv{sh}", name="xv")
                nc.sync.dma_start(xv[s][:, :, 0:h1], d_in[f"xv{s}"][:, :, 0:h1])
                if h1 < nk:
                    nc.sync.dma_start(xv[s][:, :, h1:nk], d_in[f"xv{s}"][:, :, h1:nk])

            def alloc_slot(s, qc, t):
                sh = "AC" if s in ("A", "C") else "B"
                qT[s] = pp.tile([P, NDT, qc], BF16, tag=f"qT{sh}", name="qT")
                kT[s] = pp.tile([P, NDT, t * P], BF16, tag=f"kT{sh}", name="kT")
                vv[s] = pp.tile([P, t, H, DH + 1], BF16, tag=f"v{sh}", name="vv")
                nc.vector.memset(vv[s][:, :, :, DH : DH + 1], 1.0)

            # DMA order tuned so the first projections/attention steps are
            # never waiting: wq/wk arrive per-mt-slice, x chunks interleaved
            wq_sb = pp.tile([P, NDT, D], BF16, tag="wq")
            nc.sync.dma_start(wq_sb[:], d_in["wq"][:])
            xq["A"] = xp.tile([P, NDT, 512], BF16, tag="xqAC", name="xq")
            nc.sync.dma_start(xq["A"][:, :, 0:256], d_in["xqA"][:, :, 0:256])
            nc.sync.dma_start(xq["A"][:, :, 256:512], d_in["xqA"][:, :, 256:512])
            wk_sb = pp.tile([P, NDT, D], BF16, tag="wk")
            nc.sync.dma_start(wk_sb[:], d_in["wk"][:])
            bqT_sb = pp.tile([P, NDT], F32, tag="bqT")
            nc.sync.dma_start(bqT_sb[:], d_in["bqT"][:])
            bkT_sb = pp.tile([P, NDT], F32, tag="bkT")
            nc.sync.dma_start(bkT_sb[:], d_in["bkT"][:])
            msk["A"] = pp.tile([P, T_A], F32, tag="mA", name="msk")
            nc.sync.dma_start(msk["A"][:], d_in["maskA"][:])
            nkA_ = T_A * P
            xk["A"] = xp.tile([P, NDT, nkA_], BF16, tag="xkAC", name="xk")
            nc.sync.dma_start(xk["A"][:, :, 0 : min(512, nkA_)], d_in["xkA"][:, :, 0 : min(512, nkA_)])
            wv_sb = pp.tile([P, NDT, D], BF16, tag="wv")
            nc.sync.dma_start(wv_sb[:], d_in["wv"][:])
            xv["A"] = xp.tile([P, NDT, nkA_], BF16, tag="xvAC", name="xv")
            nc.sync.dma_start(xv["A"][:, :, 0 : min(512, nkA_)], d_in["xvA"][:, :, 0 : min(512, nkA_)])
            for c0 in range(512, nkA_, 256):
                ce = min(c0 + 256, nkA_)
                nc.sync.dma_start(xk["A"][:, :, c0:ce], d_in["xkA"][:, :, c0:ce])
                nc.sync.dma_start(xv["A"][:, :, c0:ce], d_in["xvA"][:, :, c0:ce])
            bv_sb = pp.tile([P, D], BF16, tag="bv")
            nc.sync.dma_start(bv_sb[:], d_in["bvb"][:])
            for s, qc, t in slots[1:]:
                msk[s] = pp.tile([P, t], F32, tag=f"m{s}", name="msk")
                nc.sync.dma_start(msk[s][:], d_in[f"mask{s}"][:])
            load_x("B", 1024, T_B)
            wo_sb = pp.tile([P, NDT, D], BF16, tag="wo")
            nc.sync.dma_start(wo_sb[:], d_in["wo"][:])
            bo_sb = pp.tile([P, D], BF16, tag="bo")
            nc.sync.dma_start(bo_sb[:], d_in["bob"][:])

            alloc_slot("A", 512, T_A)
            alloc_slot("B", 1024, T_B)
            ctxT["A"] = pp.tile([P, NDT, 512], BF16, tag="cTA", name="ctxT")
            ctxT["B"] = pp.tile([P, NDT, 1024], BF16, tag="cTB", name="ctxT")
            ctxT["C"] = pp.tile([P, NDT, 512], BF16, tag="cTC", name="ctxT")

            # e64 selects the denominator row via PE broadcast; only used for
            # the very last flush (low-latency tail path)
            ones1_sb = pp.tile([1, P], BF16, tag="ones1")
            nc.vector.memset(ones1_sb[:], 1.0)
            e64_sb = pp.tile([DH + 1, DH], BF16, tag="e64")
            nc.vector.memset(e64_sb[:], 0.0)
            nc.vector.memset(e64_sb[DH : DH + 1, :], 1.0)

            # ---- shared PSUM pools (8 banks total) ----
            _scp_cm = tc.tile_pool(name="scps", bufs=2, space="PSUM")
            _cxp_cm = tc.tile_pool(name="cxps", bufs=2, space="PSUM")
            scp = _scp_cm.__enter__()
            cxp = _cxp_cm.__enter__()

            def proj_cols(w_sb, x_sb, o_sb, b_sb, mt, c0, w, pool, tag):
                """columns [c0:c0+w) of a projection output, mt-group (w<=1024)."""
                ps = pool.tile([P, 2, 512], F32, tag=tag, name="pj")
                nq = (w + 511) // 512
                for qc in range(nq):
                    wc = min(512, w - qc * 512)
                    for kt in range(NDT):
                        nc.tensor.matmul(
                            ps[:, qc, 0:wc],
                            lhsT=w_sb[:, kt, mt * P : (mt + 1) * P],
                            rhs=x_sb[:, kt, c0 + qc * 512 : c0 + qc * 512 + wc],
                            start=(kt == 0),
                            stop=(kt == NDT - 1),
                        )
                if w % 512 == 0:
                    nc.vector.tensor_scalar_add(
                        o_sb[:, mt, c0 : c0 + w],
                        ps[:, 0:nq, :].rearrange("p a b -> p (a b)"),
                        b_sb[:, mt : mt + 1],
                    )
                else:
                    for qc in range(nq):
                        wc = min(512, w - qc * 512)
                        nc.vector.tensor_scalar_add(
                            o_sb[:, mt, c0 + qc * 512 : c0 + qc * 512 + wc],
                            ps[:, qc, 0:wc],
                            b_sb[:, mt : mt + 1],
                        )

            def v_proj_group(s, st, pool, tag):
                psv = pool.tile([P, 2, 512], F32, tag=tag, name="pv")
                for kt in range(NDT):
                    nc.tensor.matmul(
                        psv[:, 0, :],
                        lhsT=xv[s][:, kt, st * P : (st + 1) * P],
                        rhs=wv_sb[:, kt, :],
                        start=(kt == 0),
                        stop=(kt == NDT - 1),
                    )
                nc.vector.tensor_tensor(
                    out=vv[s][:, st, :, 0:DH],
                    in0=psv[:, 0, :].rearrange("p (h d) -> p h d", h=H),
                    in1=bv_sb[:].rearrange("p (h d) -> p h d", h=H),
                    op=ADD,
                )

            # work queue of deferred thunks popped inside attention loops
            workq = []

            def pop_work(n=1):
                for _ in range(n):
                    if workq:
                        workq.pop(0)()

            with tc.tile_pool(name="ptp", bufs=8) as ptp, \
                 tc.tile_pool(name="cup", bufs=4) as cup, \
                 tc.tile_pool(name="rrp", bufs=2) as rrp, \
                 tc.tile_pool(name="rbp", bufs=2) as rbp, \
                 tc.tile_pool(name="obp", bufs=4) as obp:

                def oproj_group(s, i, g, pool, tag):
                    pso = pool.tile([P, 2, 512], F32, tag=tag, name="pso")
                    for kt in range(NDT):
                        nc.tensor.matmul(
                            pso[:, 0, :],
                            lhsT=ctxT[s][:, kt, i * P : (i + 1) * P],
                            rhs=wo_sb[:, kt, :],
                            start=(kt == 0),
                            stop=(kt == NDT - 1),
                        )
                    ot = obp.tile([P, D], BF16, tag="ot")
                    nc.vector.tensor_tensor(out=ot[:], in0=pso[:, 0, :], in1=bo_sb[:], op=ADD)
                    nc.sync.dma_start(out_d[:, g, :], ot[:])

                def flush(s, cxt, dests, last=False):
                    """Normalize one head(-pair): ctx rows / denominator row.

                    dests: list of (i, out_ap) pairs; bank i of cxt holds one
                    head's [65 x 512] ctx^T with the denominator in row 64.
                    Steady state runs entirely on DVE+Pool (no PE stall); the
                    `last` flush uses the PE broadcast + DVE multiply chain for
                    lower latency at the kernel tail.
                    """
                    cu = cup.tile([DH + 1, 2, 512], BF16, tag="cu", name="cu")
                    nc.vector.tensor_copy(cu[:], cxt[0 : DH + 1])
                    if last:
                        rb = cxp.tile([P, 2, 512], F32, tag="cx", name="rb")
                        for i, _ in dests:
                            nc.tensor.matmul(
                                rb[0:DH, i, :],
                                lhsT=e64_sb[:],
                                rhs=cu[:, i, :],
                                start=True,
                                stop=True,
                            )
                        rc = rrp.tile([DH, 2, 512], BF16, tag="rcl", name="rc", bufs=2)
                        with nc.allow_low_precision(reason="1/denom in bf16 is fine"):
                            nc.vector.reciprocal(rc[:], rb[0:DH])
                        for i, dst in dests:
                            nc.vector.tensor_tensor(
                                out=dst, in0=cu[0:DH, i, :], in1=rc[:, i, :], op=MUL
                            )
                        return
                    rc = rrp.tile([1, 2, 512], F32, tag="rc", name="rc")
                    nc.vector.reciprocal(rc[:], cu[DH : DH + 1])
                    rcb = rbp.tile([DH, 2, 512], F32, tag="rcb", name="rcb")
                    nc.gpsimd.partition_broadcast(rcb[:], rc[:])
                    for i, dst in dests:
                        nc.gpsimd.tensor_tensor(
                            out=dst, in0=cu[0:DH, i, :], in1=rcb[:, i, :], op=MUL
                        )

                def attn_pair_slot(s, T, jit_v, last=False, tail_oproj=False,
                                   pops=(0, 3, 6), fast_flush=False):
                    """512-query slots: process head pairs (2hp, 2hp+1)."""
                    pso_t = []
                    for hp in range(H // 2):
                        cxt = cxp.tile([P, 2, 512], F32, tag="cx", name="cxt")

                        def pv_step(t, pt, cxt=cxt, hp=hp):
                            for i in range(2):
                                nc.tensor.matmul(
                                    cxt[0 : DH + 1, i, :],
                                    lhsT=vv[s][:, t, 2 * hp + i, :],
                                    rhs=pt[:, i, :],
                                    start=(t == 0),
                                    stop=(t == T - 1),
                                )

                        prevs = []
                        for t in range(T):
                            if jit_v and hp == 0:
                                # V tiles are only consumed by the lag-7 PV
                                # stream, so project them 5 steps late: the xv
                                # DMA chunks get that much more arrival budget
                                if t >= 5:
                                    v_proj_group(s, t - 5, cxp, "cx")
                                if t % 4 == 3:
                                    pop_work()
                            elif t % 8 in pops:
                                pop_work()
                            sc = scp.tile([P, 2, 512], F32, tag="sc", name="sc")
                            for i in range(2):
                                nc.tensor.matmul(
                                    sc[:, i, :],
                                    lhsT=kT[s][i * 64 : i * 64 + 64, hp, t * P : (t + 1) * P],
                                    rhs=qT[s][i * 64 : i * 64 + 64, hp, 0:512],
                                    start=True,
                                    stop=True,
                                )
                            pt = ptp.tile([P, 2, 512], BF16, tag="pt", name="pt")
                            nc.scalar.activation(
                                pt[:].rearrange("p a b -> p (a b)"),
                                sc[:].rearrange("p a b -> p (a b)"),
                                Exp,
                                bias=msk[s][:, t : t + 1],
                                scale=0.125,
                            )
                            prevs.append((t, pt))
                            if len(prevs) > 7:
                                pv_step(*prevs.pop(0))
                        if jit_v and hp == 0:
                            for tt in range(max(0, T - 5), T):
                                v_proj_group(s, tt, cxp, "cx")
                        for pv in prevs:
                            pv_step(*pv)

                        def stage_half(half):
                            # output-projection partials (mt 0..2): the tail
                            # only needs the mt3 matmuls after the final flush
                            ps = scp.tile([P, 2, 512], F32, tag="sc", name="pso")
                            for g in range(2):
                                i = half * 2 + g
                                for kt in range(NDT - 1):
                                    nc.tensor.matmul(
                                        ps[:, g, :],
                                        lhsT=ctxT[s][:, kt, i * P : (i + 1) * P],
                                        rhs=wo_sb[:, kt, :],
                                        start=(kt == 0),
                                        stop=False,
                                    )
                            pso_t.append(ps)

                        if tail_oproj and hp == H // 2 - 1:
                            # half before the flush (fills the cu-copy window)
                            # and half after its rb (fills the recip window)
                            stage_half(0)

                        flush(
                            s,
                            cxt,
                            [
                                (i, ctxT[s][i * 64 : i * 64 + 64, hp, 0:512])
                                for i in range(2)
                            ],
                            last=fast_flush or (last and hp == H // 2 - 1),
                        )
                        if tail_oproj and hp == H // 2 - 1:
                            stage_half(1)

                    if tail_oproj:
                        # tail: bias via a K=1 ones-row matmul (PE is idle
                        # here) and DMA straight from PSUM -- no DVE chain
                        for half in range(2):
                            ps = pso_t[half]
                            for g in range(2):
                                i = half * 2 + g
                                nc.tensor.matmul(
                                    ps[:, g, :],
                                    lhsT=ctxT[s][:, NDT - 1, i * P : (i + 1) * P],
                                    rhs=wo_sb[:, NDT - 1, :],
                                    start=False,
                                    stop=False,
                                )
                                nc.tensor.matmul(
                                    ps[:, g, :],
                                    lhsT=ones1_sb[:],
                                    rhs=bo_sb[0:1, :],
                                    start=False,
                                    stop=True,
                                )
                                ot = obp.tile([P, D], BF16, tag="ot")
                                if g == 0:
                                    nc.scalar.activation(
                                        ot[:], ps[:, g, :],
                                        mybir.ActivationFunctionType.Copy,
                                    )
                                else:
                                    nc.vector.tensor_copy(ot[:], ps[:, g, :])
                                nc.sync.dma_start(out_d[:, 12 + i, :], ot[:])

                def attn_wide_slot(s, T, jit_v):
                    """slot B: 1024 queries, one head at a time."""
                    for h in range(H):
                        pbase = (h % 2) * 64
                        hm = h // 2
                        cxt = cxp.tile([P, 2, 512], F32, tag="cx", name="cxt")

                        def pv_step(t, pt, cxt=cxt, h=h):
                            for i in range(2):
                                nc.tensor.matmul(
                                    cxt[0 : DH + 1, i, :],
                                    lhsT=vv[s][:, t, h, :],
                                    rhs=pt[:, i, :],
                                    start=(t == 0),
                                    stop=(t == T - 1),
                                )

                        prevs = []
                        for t in range(T):
                            if jit_v and h == 0:
                                v_proj_group(s, t, cxp, "cx")
                                if t % 4 == 3:
                                    pop_work()
                            elif t % 2 == 1 or t == T - 1:
                                pop_work()
                            sc = scp.tile([P, 2, 512], F32, tag="sc", name="sc")
                            for i in range(2):
                                nc.tensor.matmul(
                                    sc[:, i, :],
                                    lhsT=kT[s][pbase : pbase + 64, hm, t * P : (t + 1) * P],
                                    rhs=qT[s][pbase : pbase + 64, hm, i * 512 : (i + 1) * 512],
                                    start=True,
                                    stop=True,
                                )
                            pt = ptp.tile([P, 2, 512], BF16, tag="pt", name="pt")
                            nc.scalar.activation(
                                pt[:].rearrange("p a b -> p (a b)"),
                                sc[:].rearrange("p a b -> p (a b)"),
                                Exp,
                                bias=msk[s][:, t : t + 1],
                                scale=0.125,
                            )
                            prevs.append((t, pt))
                            if len(prevs) > 7:
                                pv_step(*prevs.pop(0))
                        for pv in prevs:
                            pv_step(*pv)

                        flush(
                            s,
                            cxt,
                            [(0, ctxT[s][pbase : pbase + 64, hm, 0:512]),
                             (1, ctxT[s][pbase : pbase + 64, hm, 512:1024])],
                            last=(h == H - 1),
                        )

                def queue_qk_proj(s, qc, t, mts):
                    nk = t * P
                    for mt in mts:
                        for q0 in range(0, qc, 512):
                            workq.append(
                                lambda mt=mt, q0=q0: proj_cols(
                                    wq_sb, xq[s], qT[s], bqT_sb, mt, q0, min(512, qc - q0), cxp, "cx"
                                )
                            )
                        for c0 in range(0, nk, 512):
                            workq.append(
                                lambda mt=mt, c0=c0: proj_cols(
                                    wk_sb, xk[s], kT[s], bkT_sb, mt, c0, min(512, nk - c0), cxp, "cx"
                                )
                            )

                # ---- prologue: Q_A mt0/1 + first K_A groups direct; the first
                # Q/K groups are 256 wide so the PE starts as soon as possible
                nkA = T_A * P
                proj_cols(wq_sb, xq["A"], qT["A"], bqT_sb, 0, 0, 256, scp, "sc")
                proj_cols(wq_sb, xq["A"], qT["A"], bqT_sb, 0, 256, 256, scp, "sc")
                proj_cols(wq_sb, xq["A"], qT["A"], bqT_sb, 1, 0, 512, scp, "sc")
                for mt in range(2):
                    proj_cols(wk_sb, xk["A"], kT["A"], bkT_sb, mt, 0, min(512, nkA), scp, "sc")
                for mt in range(2):
                    for c0 in range(512, nkA, 512):
                        workq.append(
                            lambda mt=mt, c0=c0: proj_cols(
                                wk_sb, xk["A"], kT["A"], bkT_sb, mt, c0, min(512, nkA - c0), cxp, "cx"
                            )
                        )
                queue_qk_proj("A", 512, T_A, (2, 3))
                queue_qk_proj("B", 1024, T_B, range(NDT))

                attn_pair_slot("A", T_A, jit_v=True)

                load_x("C", 512, T_C)
                alloc_slot("C", 512, T_C)
                for t in range(T_C):
                    workq.append(lambda t=t: v_proj_group("C", t, cxp, "cx"))
                queue_qk_proj("C", 512, T_C, range(NDT))
                for i in range(4):
                    workq.append(lambda i=i: oproj_group("A", i, i, cxp, "cx"))

                attn_wide_slot("B", T_B, jit_v=True)

                for i in range(8):
                    workq.append(lambda i=i: oproj_group("B", i, 4 + i, cxp, "cx"))

                attn_pair_slot("C", T_C, jit_v=False, last=True, tail_oproj=True,
                               pops=(0, 2, 4), fast_flush=True)
                pop_work(len(workq))

            _cxp_cm.__exit__(None, None, None)
            _scp_cm.__exit__(None, None, None)

    nc.compile()
    return nc


_program_cache = {}


def _get_program(key=(16, 9, 5)):
    if key not in _program_cache:
        _program_cache[key] = _build_program(*key)
    return _program_cache[key]


def _to_bf16_T_tiled(x):
    # [n, D] fp32 -> x^T [D, n] -> [P, NDT, n] bf16 with d = dt*128 + p
    n = x.shape[0]
    xt = np.ascontiguousarray(x.T.astype(ml_dtypes.bfloat16))
    return np.ascontiguousarray(xt.reshape(NDT, P, n).transpose(1, 0, 2))


def _w_T_tiled(w):
    wt = np.ascontiguousarray(w.T.astype(ml_dtypes.bfloat16))
    return np.ascontiguousarray(wt.reshape(NDT, P, w.shape[0]).transpose(1, 0, 2))


def kernel(**inputs):
    global last_results
    x_Q = np.asarray(inputs["x_Q"], dtype=np.float32)
    x_K = np.asarray(inputs["x_K"], dtype=np.float32)
    x_V = np.asarray(inputs["x_V"], dtype=np.float32)
    Wq = np.asarray(inputs["Wq"], dtype=np.float32)
    Wk = np.asarray(inputs["Wk"], dtype=np.float32)
    Wv = np.asarray(inputs["Wv"], dtype=np.float32)
    Wo = np.asarray(inputs["Wo"], dtype=np.float32)
    bq = np.asarray(inputs["bq"], dtype=np.float32)
    bk = np.asarray(inputs["bk"], dtype=np.float32)
    bv = np.asarray(inputs["bv"], dtype=np.float32)
    bo = np.asarray(inputs["bo"], dtype=np.float32)
    lens = np.asarray(inputs["src_batch_lens"]).astype(np.int64)

    d = np.maximum(1, np.ceil(lens / P)).astype(np.int64)  # key-tiles per batch
    ranks = np.argsort(-d, kind="stable")
    T_A, T_B, T_C = int(d[ranks[0]]), int(d[ranks[2]]), int(d[ranks[6]])
    nc = _get_program((T_A, T_B, T_C))

    wqT = _w_T_tiled(Wq)
    wkT = _w_T_tiled(Wk)
    wvT = _w_T_tiled(Wv)
    woT = _w_T_tiled(Wo)
    bqT = np.ascontiguousarray(bq.reshape(NDT, P).T).astype(np.float32)
    bkT = np.ascontiguousarray(bk.reshape(NDT, P).T).astype(np.float32)
    bvb = np.ascontiguousarray(np.broadcast_to(bv, (P, D))).astype(ml_dtypes.bfloat16)
    bob = np.ascontiguousarray(np.broadcast_to(bo, (P, D))).astype(ml_dtypes.bfloat16)

    # per-core job table: slot -> (batch, q-offset, QC, T)
    jobs = []
    for c in range(B):
        jobs.append(
            {
                "A": (int(ranks[0] if c < 4 else ranks[1]), (c % 4) * 512, 512, T_A),
                "B": (int(ranks[2 + c // 2]), (c % 2) * 1024, 1024, T_B),
                "C": (int(ranks[6] if c < 4 else ranks[7]), (c % 4) * 512, 512, T_C),
            }
        )

    in_maps = []
    for c in range(B):
        m = {
            "wq": wqT,
            "wk": wkT,
            "wv": wvT,
            "wo": woT,
            "bqT": bqT,
            "bkT": bkT,
            "bvb": bvb,
            "bob": bob,
        }
        for s in ("A", "B", "C"):
            b, qoff, qc, t = jobs[c][s]
            nk = t * P
            m[f"xq{s}"] = _to_bf16_T_tiled(x_Q[b, qoff : qoff + qc])
            m[f"xk{s}"] = _to_bf16_T_tiled(x_K[b, :nk])
            m[f"xv{s}"] = _to_bf16_T_tiled(x_V[b, :nk])
            kpos = (np.arange(nk).reshape(t, P).T).astype(np.int64)  # [P, t]
            m[f"mask{s}"] = np.ascontiguousarray(
                np.where(kpos < lens[b], 0.0, NEG).astype(np.float32)
            )
        in_maps.append(m)

    res = run_bass_kernel_spmd(nc, in_maps, core_ids=list(range(B)))
    last_results = res

    out = np.empty((B, S, D), dtype=np.float32)
    for c in range(B):
        o = res.results[c]["out"]  # [P, 16, D] qtiles
        full = o.transpose(1, 0, 2).reshape(16 * P, D).astype(np.float32)
        bA, qoffA, _, _ = jobs[c]["A"]
        bB, qoffB, _, _ = jobs[c]["B"]
        bC, qoffC, _, _ = jobs[c]["C"]
        out[bA, qoffA : qoffA + 512] = full[0:512]
        out[bB, qoffB : qoffB + 1024] = full[512:1536]
        out[bC, qoffC : qoffC + 512] = full[1536:2048]
    return out
